# revision 1
# baseline (speedup 1.0000x reference)
"""GCNII-style GNN message passing on 8 Trainium2 NeuronCores (Bass/Tile).

Contract: kernel(**inputs) takes the FULL inputs (x [100000,500] f32,
edge_index [2,1600000] i32, w0 [500,128], b0 [128], conv_ws [8,128,128],
w1 [128,47], b1 [47]) and returns the FULL [100000,47] f32 log-softmax
output. Nodes are sharded across the 8 cores (12500 each, padded to
12544 = 98 blocks of 128); edges are bucketed by destination block on
the host; each layer gathers h[src] rows from a replicated bf16 table
(indirect DMA), scatter-adds via a one-hot matmul into PSUM, applies the
GCNII update, and AllGathers the new table across cores.

Self-contained: no imports from the problem directory.
"""
import math
import numpy as np
import ml_dtypes

import concourse.bass as bass
import concourse.mybir as mybir
import concourse.tile as tile
from concourse import bacc
from concourse.bass import ts as _ts
from concourse.bass_utils import run_bass_kernel_spmd
from concourse.masks import make_identity

P = 128
bf16 = mybir.dt.bfloat16
f32 = mybir.dt.float32
i32 = mybir.dt.int32

N_CORES = 8
ALPHA = 0.1
THETA = 0.5

_CACHE = {}


def _host_prep(x, edge_index, w0, b0, conv_ws, w1, b1, n_cores, alpha, theta):
    N, F = x.shape
    H = w0.shape[1]
    C = w1.shape[1]
    L = conv_ws.shape[0]
    npc_raw = N // n_cores
    assert npc_raw * n_cores == N
    nblk = (npc_raw + P - 1) // P
    npc = nblk * P
    ntab = npc * n_cores

    src = edge_index[0].astype(np.int64)
    dst = edge_index[1].astype(np.int64)
    deg = np.bincount(dst, minlength=N).astype(np.float32) + 1.0
    dinv = (1.0 / np.sqrt(deg)).astype(np.float32)
    norm_e = dinv[src] * dinv[dst]

    def rowmap(v):
        return (v // npc_raw) * npc + (v % npc_raw)

    src_row_e = rowmap(src)
    core_of = dst // npc_raw
    blk_of = (dst % npc_raw) // P
    dloc_of = (dst % npc_raw) % P

    counts = np.zeros((n_cores, nblk), dtype=np.int64)
    np.add.at(counts, (core_of, blk_of), 1)
    for b in range(nblk):
        lo = b * P
        hi = min(lo + P, npc_raw)
        counts[:, b] += hi - lo
    tiles_b = np.maximum((counts.max(axis=0) + P - 1) // P, 1)
    tot_tiles = int(tiles_b.sum())

    src_arr = np.full((n_cores, tot_tiles * P), npc_raw, dtype=np.int32)
    dst_arr = np.zeros((n_cores, tot_tiles * P), dtype=np.float32)
    nrm_arr = np.zeros((n_cores, tot_tiles * P), dtype=np.float32)

    tile_off = np.zeros(nblk + 1, dtype=np.int64)
    tile_off[1:] = np.cumsum(tiles_b)

    order = np.lexsort((blk_of, core_of))
    so = src_row_e[order]
    no = norm_e[order]
    dl = dloc_of[order]
    key = core_of[order] * nblk + blk_of[order]
    starts = np.searchsorted(key, np.arange(n_cores * nblk), side="left")
    ends = np.searchsorted(key, np.arange(n_cores * nblk), side="right")
    for c in range(n_cores):
        for b in range(nblk):
            s, e = starts[c * nblk + b], ends[c * nblk + b]
            m = e - s
            lo = b * P
            hi = min(lo + P, npc_raw)
            nself = hi - lo
            base = int(tile_off[b]) * P
            src_arr[c, base:base + m] = so[s:e]
            dst_arr[c, base:base + m] = dl[s:e]
            nrm_arr[c, base:base + m] = no[s:e]
            if nself > 0:
                gl = np.arange(lo, hi) + c * npc_raw
                j = slice(base + m, base + m + nself)
                src_arr[c, j] = rowmap(gl)
                dst_arr[c, j] = np.arange(nself)
                nrm_arr[c, j] = dinv[gl] * dinv[gl]
    # [tot_tiles*P] tile-major -> [P, tot_tiles] (edge (t,p) at t*P+p)
    src_arr = src_arr.reshape(n_cores, tot_tiles, P).transpose(0, 2, 1).copy()
    dst_arr = dst_arr.reshape(n_cores, tot_tiles, P).transpose(0, 2, 1).copy()
    nrm_arr = nrm_arr.reshape(n_cores, tot_tiles, P).transpose(0, 2, 1).copy()

    Fpad = ((F + P - 1) // P) * P
    x_arr = np.zeros((n_cores, npc, Fpad), dtype=ml_dtypes.bfloat16)
    for c in range(n_cores):
        x_arr[c, :npc_raw, :F] = x[c * npc_raw:(c + 1) * npc_raw].astype(
            ml_dtypes.bfloat16)

    w0p = np.zeros((Fpad, H), dtype=np.float32)
    w0p[:F] = w0
    w0p = np.ascontiguousarray(
        w0p.reshape(Fpad // P, P, H).transpose(1, 0, 2).reshape(P, -1)
    ).astype(ml_dtypes.bfloat16)
    b0c = b0.astype(np.float32).reshape(H, 1)

    betas = [math.log(theta / (l + 1) + 1.0) for l in range(L)]
    what = np.stack([(betas[l] / (1.0 - betas[l])) * conv_ws[l]
                     for l in range(L)])
    what = np.ascontiguousarray(
        what.transpose(1, 0, 2).reshape(H, L * H)).astype(ml_dtypes.bfloat16)
    s_scale = []
    for l in range(L):
        s = 1.0 - alpha
        if l > 0:
            s *= 1.0 - betas[l - 1]
        s_scale.append(float(s))

    Cpad = 64 if C <= 64 else ((C + P - 1) // P) * P
    w1p = np.zeros((H, Cpad), dtype=np.float32)
    w1p[:, :C] = w1 * (1.0 - betas[L - 1])
    w1p = w1p.astype(ml_dtypes.bfloat16)
    b1c = np.zeros((Cpad, 1), dtype=np.float32)
    b1c[:C, 0] = b1

    iota_np = np.tile(np.arange(P, dtype=np.float32)[None, :],
                      (P, 1)).astype(ml_dtypes.bfloat16)

    sched = dict(H=H, C=C, L=L, npc_raw=npc_raw, nblk=nblk, npc=npc,
                 ntab=ntab, Fpad=Fpad, Cpad=Cpad, tot_tiles=tot_tiles,
                 tiles_b=tiles_b.tolist(), tile_off=tile_off.tolist(),
                 s_scale=s_scale, alpha=alpha)
    percore = dict(x=x_arr, src=src_arr, dsts=dst_arr, nrm=nrm_arr)
    shared = dict(w0=w0p, b0=b0c, what=what, w1=w1p, b1=b1c, iota=iota_np)
    return sched, percore, shared


def _build_program(sched, n_cores):
    H, L = sched["H"], sched["L"]
    nblk, npc, ntab = sched["nblk"], sched["npc"], sched["ntab"]
    Fpad, Cpad, C = sched["Fpad"], sched["Cpad"], sched["C"]
    tot_tiles = sched["tot_tiles"]
    tiles_b = sched["tiles_b"]
    tile_off = sched["tile_off"]
    s_scale = sched["s_scale"]
    alpha = sched["alpha"]
    KF = Fpad // P

    nc = bacc.Bacc("TRN2", target_bir_lowering=False, debug=False,
                   num_devices=n_cores)

    x_d = nc.dram_tensor("x", [npc, Fpad], bf16, kind="ExternalInput")
    src_d = nc.dram_tensor("src", [P, tot_tiles], i32, kind="ExternalInput")
    dst_d = nc.dram_tensor("dsts", [P, tot_tiles], f32, kind="ExternalInput")
    nrm_d = nc.dram_tensor("nrm", [P, tot_tiles], f32, kind="ExternalInput")
    w0_d = nc.dram_tensor("w0", [P, KF * H], bf16, kind="ExternalInput")
    b0_d = nc.dram_tensor("b0", [H, 1], f32, kind="ExternalInput")
    wh_d = nc.dram_tensor("what", [P, L * H], bf16, kind="ExternalInput")
    w1_d = nc.dram_tensor("w1", [H, Cpad], bf16, kind="ExternalInput")
    b1_d = nc.dram_tensor("b1", [Cpad, 1], f32, kind="ExternalInput")
    iota_d = nc.dram_tensor("iota", [P, P], bf16, kind="ExternalInput")
    out_d = nc.dram_tensor("out", [npc, C], f32, kind="ExternalOutput")

    tabin = [nc.dram_tensor(f"tabin{l}", [npc, H], bf16) for l in range(L)]
    tab = [nc.dram_tensor(f"tab{l}", [ntab, H], bf16, addr_space="Shared")
           for l in range(L)]

    with tile.TileContext(nc) as tc:
        with tc.tile_pool(name="persist", bufs=1) as pp:
            src_sb = pp.tile([P, tot_tiles], i32)
            nc.sync.dma_start(out=src_sb[:], in_=src_d[:])
            dst_sb = pp.tile([P, tot_tiles], f32)
            nc.sync.dma_start(out=dst_sb[:], in_=dst_d[:])
            nrm_sb = pp.tile([P, tot_tiles], f32)
            nc.sync.dma_start(out=nrm_sb[:], in_=nrm_d[:])
            iota_sb = pp.tile([P, P], bf16)
            nc.sync.dma_start(out=iota_sb[:], in_=iota_d[:])
            w0_sb = pp.tile([P, KF * H], bf16)
            nc.sync.dma_start(out=w0_sb[:], in_=w0_d[:])
            b0_sb = pp.tile([H, 1], f32)
            nc.sync.dma_start(out=b0_sb[:], in_=b0_d[:])
            wh_sb = pp.tile([P, L * H], bf16)
            nc.sync.dma_start(out=wh_sb[:], in_=wh_d[:])
            w1_sb = pp.tile([H, Cpad], bf16)
            nc.sync.dma_start(out=w1_sb[:], in_=w1_d[:])
            b1_sb = pp.tile([Cpad, 1], f32)
            nc.sync.dma_start(out=b1_sb[:], in_=b1_d[:])
            idbf = pp.tile([P, P], bf16)
            make_identity(nc, idbf[:])
            idf32 = pp.tile([P, P], f32)
            make_identity(nc, idf32[:])
            h0s = pp.tile([P, npc], f32)

            # phase 1: h0 = relu(x@w0 + b0)
            with (
                tc.tile_pool(name="xph", bufs=3) as xp,
                tc.tile_pool(name="psX", bufs=2, space="PSUM") as psX,
            ):
                for b in range(nblk):
                    xt = xp.tile([P, Fpad], bf16, tag="xt")
                    nc.sync.dma_start(out=xt[:], in_=x_d[b * P:(b + 1) * P, :])
                    xT = xp.tile([P, Fpad], bf16, tag="xT")
                    for k in range(KF):
                        tp = psX.tile([P, P], bf16, tag="tp")
                        nc.tensor.transpose(tp[:], xt[:, _ts(k, P)], idbf[:])
                        nc.vector.tensor_copy(out=xT[:, _ts(k, P)], in_=tp[:])
                    h0p = psX.tile([P, P], f32, tag="h0p")
                    for k in range(KF):
                        nc.tensor.matmul(
                            h0p[:], w0_sb[:, _ts(k, H)], xT[:, _ts(k, P)],
                            start=(k == 0), stop=(k == KF - 1))
                    hb = xp.tile([P, P], f32, tag="hb")
                    nc.scalar.activation(hb[:], h0p[:],
                                         mybir.ActivationFunctionType.Relu,
                                         bias=b0_sb[:], scale=1.0)
                    nc.vector.tensor_scalar(
                        out=h0s[:, b * P:(b + 1) * P], in0=hb[:],
                        scalar1=alpha, scalar2=None,
                        op0=mybir.AluOpType.mult)
                    hbb = xp.tile([P, P], bf16, tag="hbb")
                    nc.vector.tensor_copy(out=hbb[:], in_=hb[:])
                    hp = psX.tile([P, P], bf16, tag="hp")
                    nc.tensor.transpose(hp[:], hbb[:], idbf[:])
                    h0n = xp.tile([P, P], bf16, tag="h0n")
                    nc.vector.tensor_copy(out=h0n[:], in_=hp[:])
                    nc.sync.dma_start(out=tabin[0][b * P:(b + 1) * P, :],
                                      in_=h0n[:])
            nc.gpsimd.collective_compute(
                "AllGather", mybir.AluOpType.bypass,
                replica_groups=[list(range(n_cores))],
                ins=[tabin[0].ap().opt()], outs=[tab[0].ap().opt()])

            # phase 2: L GCNII layers
            with (
                tc.tile_pool(name="gat", bufs=8) as gp,
                tc.tile_pool(name="epi", bufs=3) as ep,
                tc.tile_pool(name="psL", bufs=2, space="PSUM") as psL,
                tc.tile_pool(name="psM", bufs=2, space="PSUM") as psM,
            ):
                for l in range(L):
                    for b in range(nblk):
                        agg = psL.tile([P, P], f32, tag="agg")
                        nt = tiles_b[b]
                        for t in range(nt):
                            col = tile_off[b] + t
                            g = gp.tile([P, H], bf16, tag="g")
                            nc.gpsimd.indirect_dma_start(
                                out=g[:], out_offset=None, in_=tab[l][:],
                                in_offset=bass.IndirectOffsetOnAxis(
                                    ap=src_sb[:, col:col + 1], axis=0))
                            oh = gp.tile([P, P], bf16, tag="oh")
                            nc.vector.tensor_scalar(
                                out=oh[:], in0=iota_sb[:],
                                scalar1=dst_sb[:, col:col + 1],
                                scalar2=nrm_sb[:, col:col + 1],
                                op0=mybir.AluOpType.is_equal,
                                op1=mybir.AluOpType.mult)
                            nc.tensor.matmul(agg[:], g[:], oh[:],
                                             start=(t == 0),
                                             stop=(t == nt - 1))
                        zt = ep.tile([P, P], f32, tag="zt")
                        nc.vector.tensor_scalar(
                            out=zt[:], in0=agg[:], scalar1=s_scale[l],
                            scalar2=None, op0=mybir.AluOpType.mult)
                        nc.vector.tensor_tensor(
                            out=zt[:], in0=zt[:],
                            in1=h0s[:, b * P:(b + 1) * P],
                            op=mybir.AluOpType.add)
                        zb = ep.tile([P, P], bf16, tag="zb")
                        nc.vector.tensor_copy(out=zb[:], in_=zt[:])
                        zw = psL.tile([P, P], f32, tag="zw")
                        nc.tensor.matmul(zw[:], wh_sb[:, _ts(l, H)], zb[:],
                                         start=True, stop=True)
                        hs = ep.tile([P, P], f32, tag="hs")
                        nc.vector.tensor_tensor(out=hs[:], in0=zt[:],
                                                in1=zw[:],
                                                op=mybir.AluOpType.add)
                        hn = ep.tile([P, P], bf16, tag="hn")
                        nc.scalar.activation(
                            hn[:], hs[:], mybir.ActivationFunctionType.Relu)
                        if l < L - 1:
                            tr = psM.tile([P, P], bf16, tag="tr")
                            nc.tensor.transpose(tr[:], hn[:], idbf[:])
                            hb2 = ep.tile([P, P], bf16, tag="hb2")
                            nc.vector.tensor_copy(out=hb2[:], in_=tr[:])
                            nc.sync.dma_start(
                                out=tabin[l + 1][b * P:(b + 1) * P, :],
                                in_=hb2[:])
                        else:
                            lg = psM.tile([P, P], f32, tag="tr")
                            nc.tensor.matmul(lg[:Cpad, :], w1_sb[:], hn[:],
                                             start=True, stop=True)
                            lgb = ep.tile([P, P], f32, tag="lgb")
                            nc.vector.tensor_scalar(
                                out=lgb[:Cpad, :], in0=lg[:Cpad, :],
                                scalar1=b1_sb[:Cpad, :1], scalar2=None,
                                op0=mybir.AluOpType.add)
                            lt = psM.tile([P, P], f32, tag="tr")
                            nc.tensor.transpose(lt[:], lgb[:], idf32[:])
                            lts = ep.tile([P, P], f32, tag="lts")
                            nc.vector.tensor_copy(out=lts[:], in_=lt[:])
                            mx = ep.tile([P, 1], f32, tag="mx")
                            nc.vector.reduce_max(mx[:], lts[:, :C],
                                                 axis=mybir.AxisListType.X)
                            sh = ep.tile([P, P], f32, tag="sh")
                            nc.vector.tensor_scalar(
                                out=sh[:, :C], in0=lts[:, :C],
                                scalar1=mx[:], scalar2=None,
                                op0=mybir.AluOpType.subtract)
                            exp_t = ep.tile([P, P], f32, tag="exp")
                            nc.scalar.activation(
                                exp_t[:, :C], sh[:, :C],
                                mybir.ActivationFunctionType.Exp)
                            sm = ep.tile([P, 1], f32, tag="sm")
                            nc.vector.reduce_sum(sm[:], exp_t[:, :C],
                                                 axis=mybir.AxisListType.X)
                            lnsm = ep.tile([P, 1], f32, tag="lnsm")
                            nc.scalar.activation(
                                lnsm[:], sm[:],
                                mybir.ActivationFunctionType.Ln)
                            fin = ep.tile([P, P], f32, tag="fin")
                            nc.vector.tensor_scalar(
                                out=fin[:, :C], in0=sh[:, :C],
                                scalar1=lnsm[:], scalar2=None,
                                op0=mybir.AluOpType.subtract)
                            nc.sync.dma_start(
                                out=out_d[b * P:(b + 1) * P, :],
                                in_=fin[:, :C])
                    if l < L - 1:
                        nc.gpsimd.collective_compute(
                            "AllGather", mybir.AluOpType.bypass,
                            replica_groups=[list(range(n_cores))],
                            ins=[tabin[l + 1].ap().opt()],
                            outs=[tab[l + 1].ap().opt()])

    nc.compile()
    return nc


def kernel(x, edge_index, w0, b0, conv_ws, w1, b1):
    x = np.asarray(x, dtype=np.float32)
    edge_index = np.asarray(edge_index)
    w0 = np.asarray(w0, dtype=np.float32)
    b0 = np.asarray(b0, dtype=np.float32)
    conv_ws = np.asarray(conv_ws, dtype=np.float32)
    w1 = np.asarray(w1, dtype=np.float32)
    b1 = np.asarray(b1, dtype=np.float32)

    sched, percore, shared = _host_prep(
        x, edge_index, w0, b0, conv_ws, w1, b1, N_CORES, ALPHA, THETA)

    key = (x.shape, edge_index.shape, sched["tot_tiles"])
    if key not in _CACHE:
        _CACHE[key] = _build_program(sched, N_CORES)
    nc = _CACHE[key]

    in_maps = []
    for c in range(N_CORES):
        in_maps.append({
            "x": percore["x"][c],
            "src": percore["src"][c],
            "dsts": percore["dsts"][c],
            "nrm": percore["nrm"][c],
            "w0": shared["w0"], "b0": shared["b0"], "what": shared["what"],
            "w1": shared["w1"], "b1": shared["b1"], "iota": shared["iota"],
        })
    res = run_bass_kernel_spmd(nc, in_maps, core_ids=list(range(N_CORES)))
    npc_raw = sched["npc_raw"]
    outs = [res.results[c]["out"][:npc_raw] for c in range(N_CORES)]
    return np.concatenate(outs, axis=0).astype(np.float32)


# revision 2
# speedup vs baseline: 1.1817x; 1.1817x over previous
"""GCNII-style GNN message passing on 8 Trainium2 NeuronCores (Bass/Tile).

Contract: kernel(**inputs) takes the FULL inputs (x [100000,500] f32,
edge_index [2,1600000] i32, w0 [500,128], b0 [128], conv_ws [8,128,128],
w1 [128,47], b1 [47]) and returns the FULL [100000,47] f32 log-softmax
output. Nodes are sharded across the 8 cores (12500 each, padded to
12544 = 98 blocks of 128); edges are bucketed by destination block on
the host; each layer gathers h[src] rows from a replicated bf16 table
(indirect DMA), scatter-adds via a one-hot matmul into PSUM, applies the
GCNII update, and AllGathers the new table across cores.

Self-contained: no imports from the problem directory.
"""
import math
import time as _time
import numpy as np
import ml_dtypes

import concourse.bass as bass
import concourse.mybir as mybir
import concourse.tile as tile
from concourse import bacc
from concourse.bass import ts as _ts
from concourse.bass_utils import run_bass_kernel_spmd
from concourse.masks import make_identity

P = 128
bf16 = mybir.dt.bfloat16
f32 = mybir.dt.float32
i32 = mybir.dt.int32

N_CORES = 8
ALPHA = 0.1
THETA = 0.5

_CACHE = {}
LAST_RUN_S = None


def _host_prep(x, edge_index, w0, b0, conv_ws, w1, b1, n_cores, alpha, theta):
    N, F = x.shape
    H = w0.shape[1]
    C = w1.shape[1]
    L = conv_ws.shape[0]
    npc_raw = N // n_cores
    assert npc_raw * n_cores == N
    nblk = (npc_raw + P - 1) // P
    npc = nblk * P
    ntab = npc * n_cores

    src = edge_index[0].astype(np.int64)
    dst = edge_index[1].astype(np.int64)
    deg = np.bincount(dst, minlength=N).astype(np.float32) + 1.0
    dinv = (1.0 / np.sqrt(deg)).astype(np.float32)
    norm_e = dinv[src] * dinv[dst]

    def rowmap(v):
        return (v // npc_raw) * npc + (v % npc_raw)

    src_row_e = rowmap(src)
    core_of = dst // npc_raw
    blk_of = (dst % npc_raw) // P
    dloc_of = (dst % npc_raw) % P

    counts = np.zeros((n_cores, nblk), dtype=np.int64)
    np.add.at(counts, (core_of, blk_of), 1)
    for b in range(nblk):
        lo = b * P
        hi = min(lo + P, npc_raw)
        counts[:, b] += hi - lo
    tiles_b = np.maximum((counts.max(axis=0) + P - 1) // P, 1)
    tot_tiles = int(tiles_b.sum())

    src_arr = np.full((n_cores, tot_tiles * P), npc_raw, dtype=np.int32)
    dst_arr = np.zeros((n_cores, tot_tiles * P), dtype=np.float32)
    nrm_arr = np.zeros((n_cores, tot_tiles * P), dtype=np.float32)

    tile_off = np.zeros(nblk + 1, dtype=np.int64)
    tile_off[1:] = np.cumsum(tiles_b)

    order = np.lexsort((blk_of, core_of))
    so = src_row_e[order]
    no = norm_e[order]
    dl = dloc_of[order]
    key = core_of[order] * nblk + blk_of[order]
    starts = np.searchsorted(key, np.arange(n_cores * nblk), side="left")
    ends = np.searchsorted(key, np.arange(n_cores * nblk), side="right")
    for c in range(n_cores):
        for b in range(nblk):
            s, e = starts[c * nblk + b], ends[c * nblk + b]
            m = e - s
            lo = b * P
            hi = min(lo + P, npc_raw)
            nself = hi - lo
            base = int(tile_off[b]) * P
            src_arr[c, base:base + m] = so[s:e]
            dst_arr[c, base:base + m] = dl[s:e]
            nrm_arr[c, base:base + m] = no[s:e]
            if nself > 0:
                gl = np.arange(lo, hi) + c * npc_raw
                j = slice(base + m, base + m + nself)
                src_arr[c, j] = rowmap(gl)
                dst_arr[c, j] = np.arange(nself)
                nrm_arr[c, j] = dinv[gl] * dinv[gl]
    # [tot_tiles*P] tile-major -> [P, tot_tiles] (edge (t,p) at t*P+p)
    src_arr = src_arr.reshape(n_cores, tot_tiles, P).transpose(0, 2, 1).copy()
    dst_arr = dst_arr.reshape(n_cores, tot_tiles, P).transpose(0, 2, 1).copy()
    nrm_arr = nrm_arr.reshape(n_cores, tot_tiles, P).transpose(0, 2, 1).copy()

    Fpad = ((F + P - 1) // P) * P
    x_arr = np.zeros((n_cores, npc, Fpad), dtype=ml_dtypes.bfloat16)
    for c in range(n_cores):
        x_arr[c, :npc_raw, :F] = x[c * npc_raw:(c + 1) * npc_raw].astype(
            ml_dtypes.bfloat16)

    w0p = np.zeros((Fpad, H), dtype=np.float32)
    w0p[:F] = w0
    w0p = np.ascontiguousarray(
        w0p.reshape(Fpad // P, P, H).transpose(1, 0, 2).reshape(P, -1)
    ).astype(ml_dtypes.bfloat16)
    b0c = b0.astype(np.float32).reshape(H, 1)

    betas = [math.log(theta / (l + 1) + 1.0) for l in range(L)]
    what = np.stack([(betas[l] / (1.0 - betas[l])) * conv_ws[l]
                     for l in range(L)])
    what = np.ascontiguousarray(
        what.transpose(1, 0, 2).reshape(H, L * H)).astype(ml_dtypes.bfloat16)
    s_scale = []
    for l in range(L):
        s = 1.0 - alpha
        if l > 0:
            s *= 1.0 - betas[l - 1]
        s_scale.append(float(s))

    Cpad = 64 if C <= 64 else ((C + P - 1) // P) * P
    w1p = np.zeros((H, Cpad), dtype=np.float32)
    w1p[:, :C] = w1 * (1.0 - betas[L - 1])
    w1p = w1p.astype(ml_dtypes.bfloat16)
    b1c = np.zeros((Cpad, 1), dtype=np.float32)
    b1c[:C, 0] = b1

    iota_np = np.tile(np.arange(P, dtype=np.float32)[None, :],
                      (P, 1)).astype(ml_dtypes.bfloat16)

    sched = dict(H=H, C=C, L=L, npc_raw=npc_raw, nblk=nblk, npc=npc,
                 ntab=ntab, Fpad=Fpad, Cpad=Cpad, tot_tiles=tot_tiles,
                 tiles_b=tiles_b.tolist(), tile_off=tile_off.tolist(),
                 s_scale=s_scale, alpha=alpha)
    percore = dict(x=x_arr, src=src_arr, dsts=dst_arr, nrm=nrm_arr)
    shared = dict(w0=w0p, b0=b0c, what=what, w1=w1p, b1=b1c, iota=iota_np)
    return sched, percore, shared


def _build_program(sched, n_cores):
    H, L = sched["H"], sched["L"]
    nblk, npc, ntab = sched["nblk"], sched["npc"], sched["ntab"]
    Fpad, Cpad, C = sched["Fpad"], sched["Cpad"], sched["C"]
    tot_tiles = sched["tot_tiles"]
    tiles_b = sched["tiles_b"]
    tile_off = sched["tile_off"]
    s_scale = sched["s_scale"]
    alpha = sched["alpha"]
    KF = Fpad // P

    nc = bacc.Bacc("TRN2", target_bir_lowering=False, debug=False,
                   num_devices=n_cores)

    x_d = nc.dram_tensor("x", [npc, Fpad], bf16, kind="ExternalInput")
    src_d = nc.dram_tensor("src", [P, tot_tiles], i32, kind="ExternalInput")
    dst_d = nc.dram_tensor("dsts", [P, tot_tiles], f32, kind="ExternalInput")
    nrm_d = nc.dram_tensor("nrm", [P, tot_tiles], f32, kind="ExternalInput")
    w0_d = nc.dram_tensor("w0", [P, KF * H], bf16, kind="ExternalInput")
    b0_d = nc.dram_tensor("b0", [H, 1], f32, kind="ExternalInput")
    wh_d = nc.dram_tensor("what", [P, L * H], bf16, kind="ExternalInput")
    w1_d = nc.dram_tensor("w1", [H, Cpad], bf16, kind="ExternalInput")
    b1_d = nc.dram_tensor("b1", [Cpad, 1], f32, kind="ExternalInput")
    iota_d = nc.dram_tensor("iota", [P, P], bf16, kind="ExternalInput")
    out_d = nc.dram_tensor("out", [npc, C], f32, kind="ExternalOutput")

    tabin = [nc.dram_tensor(f"tabin{l}", [npc, H], bf16) for l in range(L)]
    tab = [nc.dram_tensor(f"tab{l}", [ntab, H], bf16, addr_space="Shared")
           for l in range(L)]

    with tile.TileContext(nc) as tc:
        with tc.tile_pool(name="persist", bufs=1) as pp:
            src_sb = pp.tile([P, tot_tiles], i32)
            nc.sync.dma_start(out=src_sb[:], in_=src_d[:])
            dst_sb = pp.tile([P, tot_tiles], f32)
            nc.sync.dma_start(out=dst_sb[:], in_=dst_d[:])
            nrm_sb = pp.tile([P, tot_tiles], f32)
            nc.sync.dma_start(out=nrm_sb[:], in_=nrm_d[:])
            iota_sb = pp.tile([P, P], bf16)
            nc.sync.dma_start(out=iota_sb[:], in_=iota_d[:])
            w0_sb = pp.tile([P, KF * H], bf16)
            nc.sync.dma_start(out=w0_sb[:], in_=w0_d[:])
            b0_sb = pp.tile([H, 1], f32)
            nc.sync.dma_start(out=b0_sb[:], in_=b0_d[:])
            wh_sb = pp.tile([P, L * H], bf16)
            nc.sync.dma_start(out=wh_sb[:], in_=wh_d[:])
            w1_sb = pp.tile([H, Cpad], bf16)
            nc.sync.dma_start(out=w1_sb[:], in_=w1_d[:])
            b1_sb = pp.tile([Cpad, 1], f32)
            nc.sync.dma_start(out=b1_sb[:], in_=b1_d[:])
            idbf = pp.tile([P, P], bf16)
            make_identity(nc, idbf[:])
            idf32 = pp.tile([P, P], f32)
            make_identity(nc, idf32[:])
            h0s = pp.tile([P, npc], f32)

            # phase 1: h0 = relu(x@w0 + b0)
            with (
                tc.tile_pool(name="xph", bufs=3) as xp,
                tc.tile_pool(name="psX", bufs=2, space="PSUM") as psX,
            ):
                for b in range(nblk):
                    xt = xp.tile([P, Fpad], bf16, tag="xt")
                    nc.sync.dma_start(out=xt[:], in_=x_d[b * P:(b + 1) * P, :])
                    xT = xp.tile([P, Fpad], bf16, tag="xT")
                    for k in range(KF):
                        tp = psX.tile([P, P], bf16, tag="tp")
                        nc.tensor.transpose(tp[:], xt[:, _ts(k, P)], idbf[:])
                        nc.vector.tensor_copy(out=xT[:, _ts(k, P)], in_=tp[:])
                    h0p = psX.tile([P, P], f32, tag="h0p")
                    for k in range(KF):
                        nc.tensor.matmul(
                            h0p[:], w0_sb[:, _ts(k, H)], xT[:, _ts(k, P)],
                            start=(k == 0), stop=(k == KF - 1))
                    hb = xp.tile([P, P], f32, tag="hb")
                    nc.scalar.activation(hb[:], h0p[:],
                                         mybir.ActivationFunctionType.Relu,
                                         bias=b0_sb[:], scale=1.0)
                    nc.vector.tensor_scalar(
                        out=h0s[:, b * P:(b + 1) * P], in0=hb[:],
                        scalar1=alpha, scalar2=None,
                        op0=mybir.AluOpType.mult)
                    hbb = xp.tile([P, P], bf16, tag="hbb")
                    nc.vector.tensor_copy(out=hbb[:], in_=hb[:])
                    hp = psX.tile([P, P], bf16, tag="hp")
                    nc.tensor.transpose(hp[:], hbb[:], idbf[:])
                    h0n = xp.tile([P, P], bf16, tag="h0n")
                    nc.vector.tensor_copy(out=h0n[:], in_=hp[:])
                    nc.sync.dma_start(out=tabin[0][b * P:(b + 1) * P, :],
                                      in_=h0n[:])
            nc.gpsimd.collective_compute(
                "AllGather", mybir.AluOpType.bypass,
                replica_groups=[list(range(n_cores))],
                ins=[tabin[0].ap().opt()], outs=[tab[0].ap().opt()])

            # phase 2: L GCNII layers
            with (
                tc.tile_pool(name="gat", bufs=8) as gp,
                tc.tile_pool(name="epi", bufs=3) as ep,
                tc.tile_pool(name="psL", bufs=2, space="PSUM") as psL,
                tc.tile_pool(name="psM", bufs=2, space="PSUM") as psM,
            ):
                for l in range(L):
                    for b in range(nblk):
                        agg = psL.tile([P, P], f32, tag="agg")
                        nt = tiles_b[b]
                        for t in range(nt):
                            col = tile_off[b] + t
                            g = gp.tile([P, H], bf16, tag="g")
                            nc.gpsimd.indirect_dma_start(
                                out=g[:], out_offset=None, in_=tab[l][:],
                                in_offset=bass.IndirectOffsetOnAxis(
                                    ap=src_sb[:, col:col + 1], axis=0))
                            oh = gp.tile([P, P], bf16, tag="oh")
                            nc.vector.tensor_scalar(
                                out=oh[:], in0=iota_sb[:],
                                scalar1=dst_sb[:, col:col + 1],
                                scalar2=nrm_sb[:, col:col + 1],
                                op0=mybir.AluOpType.is_equal,
                                op1=mybir.AluOpType.mult)
                            nc.tensor.matmul(agg[:], g[:], oh[:],
                                             start=(t == 0),
                                             stop=(t == nt - 1))
                        zt = ep.tile([P, P], f32, tag="zt")
                        nc.vector.tensor_scalar(
                            out=zt[:], in0=agg[:], scalar1=s_scale[l],
                            scalar2=None, op0=mybir.AluOpType.mult)
                        nc.vector.tensor_tensor(
                            out=zt[:], in0=zt[:],
                            in1=h0s[:, b * P:(b + 1) * P],
                            op=mybir.AluOpType.add)
                        zb = ep.tile([P, P], bf16, tag="zb")
                        nc.vector.tensor_copy(out=zb[:], in_=zt[:])
                        zw = psL.tile([P, P], f32, tag="zw")
                        nc.tensor.matmul(zw[:], wh_sb[:, _ts(l, H)], zb[:],
                                         start=True, stop=True)
                        hs = ep.tile([P, P], f32, tag="hs")
                        nc.vector.tensor_tensor(out=hs[:], in0=zt[:],
                                                in1=zw[:],
                                                op=mybir.AluOpType.add)
                        hn = ep.tile([P, P], bf16, tag="hn")
                        nc.scalar.activation(
                            hn[:], hs[:], mybir.ActivationFunctionType.Relu)
                        if l < L - 1:
                            tr = psM.tile([P, P], bf16, tag="tr")
                            nc.tensor.transpose(tr[:], hn[:], idbf[:])
                            hb2 = ep.tile([P, P], bf16, tag="hb2")
                            nc.vector.tensor_copy(out=hb2[:], in_=tr[:])
                            nc.sync.dma_start(
                                out=tabin[l + 1][b * P:(b + 1) * P, :],
                                in_=hb2[:])
                        else:
                            lg = psM.tile([P, P], f32, tag="tr")
                            nc.tensor.matmul(lg[:Cpad, :], w1_sb[:], hn[:],
                                             start=True, stop=True)
                            lgb = ep.tile([P, P], f32, tag="lgb")
                            nc.vector.tensor_scalar(
                                out=lgb[:Cpad, :], in0=lg[:Cpad, :],
                                scalar1=b1_sb[:Cpad, :1], scalar2=None,
                                op0=mybir.AluOpType.add)
                            lt = psM.tile([P, P], f32, tag="tr")
                            nc.tensor.transpose(lt[:], lgb[:], idf32[:])
                            lts = ep.tile([P, P], f32, tag="lts")
                            nc.vector.tensor_copy(out=lts[:], in_=lt[:])
                            mx = ep.tile([P, 1], f32, tag="mx")
                            nc.vector.reduce_max(mx[:], lts[:, :C],
                                                 axis=mybir.AxisListType.X)
                            sh = ep.tile([P, P], f32, tag="sh")
                            nc.vector.tensor_scalar(
                                out=sh[:, :C], in0=lts[:, :C],
                                scalar1=mx[:], scalar2=None,
                                op0=mybir.AluOpType.subtract)
                            exp_t = ep.tile([P, P], f32, tag="exp")
                            nc.scalar.activation(
                                exp_t[:, :C], sh[:, :C],
                                mybir.ActivationFunctionType.Exp)
                            sm = ep.tile([P, 1], f32, tag="sm")
                            nc.vector.reduce_sum(sm[:], exp_t[:, :C],
                                                 axis=mybir.AxisListType.X)
                            lnsm = ep.tile([P, 1], f32, tag="lnsm")
                            nc.scalar.activation(
                                lnsm[:], sm[:],
                                mybir.ActivationFunctionType.Ln)
                            fin = ep.tile([P, P], f32, tag="fin")
                            nc.vector.tensor_scalar(
                                out=fin[:, :C], in0=sh[:, :C],
                                scalar1=lnsm[:], scalar2=None,
                                op0=mybir.AluOpType.subtract)
                            nc.sync.dma_start(
                                out=out_d[b * P:(b + 1) * P, :],
                                in_=fin[:, :C])
                    if l < L - 1:
                        nc.gpsimd.collective_compute(
                            "AllGather", mybir.AluOpType.bypass,
                            replica_groups=[list(range(n_cores))],
                            ins=[tabin[l + 1].ap().opt()],
                            outs=[tab[l + 1].ap().opt()])

    nc.compile()
    return nc


def kernel(x, edge_index, w0, b0, conv_ws, w1, b1):
    x = np.asarray(x, dtype=np.float32)
    edge_index = np.asarray(edge_index)
    w0 = np.asarray(w0, dtype=np.float32)
    b0 = np.asarray(b0, dtype=np.float32)
    conv_ws = np.asarray(conv_ws, dtype=np.float32)
    w1 = np.asarray(w1, dtype=np.float32)
    b1 = np.asarray(b1, dtype=np.float32)

    sched, percore, shared = _host_prep(
        x, edge_index, w0, b0, conv_ws, w1, b1, N_CORES, ALPHA, THETA)

    key = (x.shape, edge_index.shape, sched["tot_tiles"])
    if key not in _CACHE:
        _CACHE[key] = _build_program(sched, N_CORES)
    nc = _CACHE[key]

    in_maps = []
    for c in range(N_CORES):
        in_maps.append({
            "x": percore["x"][c],
            "src": percore["src"][c],
            "dsts": percore["dsts"][c],
            "nrm": percore["nrm"][c],
            "w0": shared["w0"], "b0": shared["b0"], "what": shared["what"],
            "w1": shared["w1"], "b1": shared["b1"], "iota": shared["iota"],
        })
    global LAST_RUN_S
    _t0 = _time.time()
    res = run_bass_kernel_spmd(nc, in_maps, core_ids=list(range(N_CORES)))
    LAST_RUN_S = _time.time() - _t0
    npc_raw = sched["npc_raw"]
    outs = [res.results[c]["out"][:npc_raw] for c in range(N_CORES)]
    return np.concatenate(outs, axis=0).astype(np.float32)
